# revision 1
# baseline (speedup 1.0000x reference)
"""AccFlowEncoder TRN2 kernel.

Dynamic voxelization of two point-cloud frames into a 512x512 pillar grid
(segment-mean of relu(feats @ W + b)); output = (tgt - src) +
time_feat * occupied, shape [2, 512, 512, 64] fp32 (128 MiB).

Sharding: 8 cores; core c owns (batch c//4, gx rows [128*(c%4), +128)) --
a [128, 512, 64] slice = 65536 pillars = 16.8 MB.

Device algorithm (raw Bass, manual semaphores -- this toolchain limits
every DMA queue entry to one semaphore wait, and its extended scatter-add
ucode library cannot be loaded, so all "scatter" is done on the PE):
  The host sorts each core's points (both frames together) by local pillar
  id and packs them into token tiles of 128 aligned to pillar tiles of 128
  consecutive pillars (2 token tiles per pillar tile; verified max 248
  tokens/tile). Per pillar tile r:
    PE : z[t,c] = featsT[9,128].T @ W9[9,64]   (fp32; feats pre-scaled by
         1/count so segment-mean becomes segment-sum; bias via 9th feature)
    ACT: h = Relu(z)  PSUM->SBUF
    DVE: hi = bf16(h*sign);  lo = bf16(h*sign - hi)   (sign = -1 frame0,
         +1 frame1;  hi+lo == h*sign to ~2^-18)
    DVE: O[t,p] = (pid_local[t] == p)  vs an iota row matrix, bf16 one-hot
    PE : grid_psum[p,c] += O.T @ hi + O.T @ lo   (bf16 matmuls, fp32 PSUM
         accumulate -> exact fp32 one-hot place-and-sum)
    ACT: grid_psum -> dense SBUF chunk
    DVE: chunk += occupied (x) time_feat   (rank-1 broadcast, per chunk)
  Dense 512 KB chunks stream to the (pre-zeroed) output with plain DMAs.
"""

import numpy as np

import concourse.bass as bass
import concourse.mybir as mybir
from concourse.bass_utils import run_bass_kernel_spmd

VX = VY = 0.2
XMIN = YMIN = -51.2
GX = GY = 512
C = 64
B = 2
N_CORES = 8
QROWS = GX // 4          # gx rows per core
NPIL = QROWS * GY        # pillars per core slice (65536)
NTILE = NPIL // 128      # pillar tiles per core (512)
CHUNK = 16               # pillar tiles per output chunk
NCHUNK = NTILE // CHUNK  # 32

_PROGRAM_CACHE = {}


def _host_prep(pc0, pc1, tpt):
    """Sort/route points; build per-core device input arrays."""
    ncols = NTILE * tpt * 128
    auxw = CHUNK * tpt
    cores = []
    for core in range(N_CORES):
        b, q = core // 4, core % 4
        feats_l, pid_l, sign_l = [], [], []
        occ_cnt = np.zeros(NPIL, np.int64)
        for f, pc in enumerate((pc0, pc1)):
            pts = pc[b]
            ix = np.clip(np.floor((pts[:, 0] - XMIN) / VX).astype(np.int64), 0, GX - 1)
            iy = np.clip(np.floor((pts[:, 1] - YMIN) / VY).astype(np.int64), 0, GY - 1)
            m = (ix // QROWS) == q
            p, ixm, iym = pts[m], ix[m], iy[m]
            pid = (ixm - QROWS * q) * GY + iym
            cnt = np.bincount(pid, minlength=NPIL).astype(np.float32)
            occ_cnt += cnt.astype(np.int64)
            sums = np.zeros((NPIL, 3), np.float64)
            np.add.at(sums, pid, p.astype(np.float64))
            mean = (sums / np.maximum(cnt, 1.0)[:, None].astype(np.float64)).astype(
                np.float32
            )
            cx = (XMIN + (ixm.astype(np.float32) + 0.5) * VX)
            cy = (YMIN + (iym.astype(np.float32) + 0.5) * VY)
            f9 = np.concatenate(
                [
                    p,
                    p - mean[pid],
                    (p[:, 0] - cx)[:, None],
                    (p[:, 1] - cy)[:, None],
                    np.ones((len(p), 1), np.float32),
                ],
                axis=1,
            )
            s = (1.0 / cnt[pid]).astype(np.float32)
            feats_l.append(f9 * s[:, None])
            pid_l.append(pid)
            sign_l.append(np.full(len(p), -1.0 if f == 0 else 1.0, np.float32))
        feats = np.concatenate(feats_l, axis=0).astype(np.float32)
        pid = np.concatenate(pid_l)
        sign = np.concatenate(sign_l)

        order = np.argsort(pid, kind="stable")
        feats, pid, sign = feats[order], pid[order], sign[order]
        tile = pid // 128

        featsT = np.zeros((9, ncols), np.float32)
        pidloc = np.full((128, NTILE * tpt), -1.0, np.float32)
        signs = np.zeros((128, NTILE * tpt), np.float32)
        start = np.searchsorted(tile, np.arange(NTILE))
        end = np.searchsorted(tile, np.arange(NTILE), side="right")
        nmax = int((end - start).max())
        assert nmax <= tpt * 128, f"a pillar tile has {nmax} tokens > {tpt * 128}"
        # rank of each token within its tile (tokens are tile-sorted)
        j = np.arange(len(pid)) - start[tile]
        tt, slot = j // 128, j % 128
        tcol = tile * tpt + tt
        featsT[:, tcol * 128 + slot] = feats.T
        pidloc[slot, tcol] = (pid - tile * 128).astype(np.float32)
        signs[slot, tcol] = sign

        aux = np.zeros((128, NCHUNK * 2 * auxw), np.float32)
        for g in range(NCHUNK):
            aux[:, g * 2 * auxw : g * 2 * auxw + auxw] = pidloc[
                :, g * auxw : (g + 1) * auxw
            ]
            aux[:, g * 2 * auxw + auxw : (g + 1) * 2 * auxw] = signs[
                :, g * auxw : (g + 1) * auxw
            ]

        occ = (occ_cnt > 0).astype(np.float32).reshape(NTILE, 128).T.copy()
        cores.append({"featsT": featsT, "aux": aux, "occ": occ})
    return cores


def _build_program(tpt):
    dt = mybir.dt
    ncols = NTILE * tpt * 128
    auxw = CHUNK * tpt
    fchunk = CHUNK * tpt * 128
    NT = NTILE
    Relu = mybir.ActivationFunctionType.Relu
    Ident = mybir.ActivationFunctionType.Copy

    nc = bass.Bass()
    feats_d = nc.dram_tensor("featsT", [9, ncols], dt.float32, kind="ExternalInput")
    aux_d = nc.dram_tensor(
        "aux", [128, NCHUNK * 2 * auxw], dt.float32, kind="ExternalInput"
    )
    occ_d = nc.dram_tensor("occ", [128, NTILE], dt.float32, kind="ExternalInput")
    w9_d = nc.dram_tensor("w9", [9, C], dt.float32, kind="ExternalInput")
    tf_d = nc.dram_tensor("tf", [128, C], dt.float32, kind="ExternalInput")
    iota_d = nc.dram_tensor("iota", [128, 128], dt.float32, kind="ExternalInput")
    out_d = nc.dram_tensor("out", [NPIL, C], dt.float32, kind="ExternalOutput")

    from contextlib import ExitStack

    with ExitStack() as ctx:
        feats_sb = ctx.enter_context(nc.sbuf_tensor([9, 2 * fchunk], dt.float32))
        aux_sb = ctx.enter_context(nc.sbuf_tensor([128, 2 * 2 * auxw], dt.float32))
        occ_sb = ctx.enter_context(nc.sbuf_tensor([128, NTILE], dt.float32))
        w9_sb = ctx.enter_context(nc.sbuf_tensor([9, C], dt.float32))
        tf_sb = ctx.enter_context(nc.sbuf_tensor([128, C], dt.float32))
        iota_sb = ctx.enter_context(nc.sbuf_tensor([128, 128], dt.float32))
        h_sb = ctx.enter_context(nc.sbuf_tensor([128, 2 * tpt * C], dt.float32))
        hi_sb = ctx.enter_context(nc.sbuf_tensor([128, 2 * tpt * C], dt.bfloat16))
        lo_sb = ctx.enter_context(nc.sbuf_tensor([128, 2 * tpt * C], dt.bfloat16))
        oh_sb = ctx.enter_context(nc.sbuf_tensor([128, 2 * tpt * 128], dt.bfloat16))
        chunk_sb = ctx.enter_context(nc.sbuf_tensor([128, 2 * CHUNK * C], dt.float32))
        tfocc_sb = ctx.enter_context(nc.sbuf_tensor([128, CHUNK * C], dt.float32))
        zps = ctx.enter_context(nc.psum_tensor([128, 2 * tpt * C], dt.float32))
        gps = ctx.enter_context(nc.psum_tensor([128, 2 * C], dt.float32))
        s_load = ctx.enter_context(nc.semaphore("s_load"))
        s_z = ctx.enter_context(nc.semaphore("s_z"))
        s_h = ctx.enter_context(nc.semaphore("s_h"))
        s_dve = ctx.enter_context(nc.semaphore("s_dve"))
        s_smm = ctx.enter_context(nc.semaphore("s_smm"))
        s_gc = ctx.enter_context(nc.semaphore("s_gc"))
        s_tf = ctx.enter_context(nc.semaphore("s_tf"))
        s_out = ctx.enter_context(nc.semaphore("s_out"))
        block = ctx.enter_context(nc.Block())

        def out_chunk_dma(sync, go):
            osl = go % 2
            sync.dma_start(
                out=out_d[go * CHUNK * 128 : (go + 1) * CHUNK * 128, :].rearrange(
                    "(t p) c -> p t c", p=128
                ),
                in_=chunk_sb[:, osl * CHUNK * C : (osl + 1) * CHUNK * C].rearrange(
                    "p (t c) -> p t c", c=C
                ),
            )._wait_ge(s_tf, go + 1).then_inc(s_out, 16)

        @block.sync
        def _(sync):
            sync.dma_start(out=w9_sb[:], in_=w9_d[:]).then_inc(s_load, 16)
            sync.dma_start(out=tf_sb[:], in_=tf_d[:]).then_inc(s_load, 16)
            sync.dma_start(out=iota_sb[:], in_=iota_d[:]).then_inc(s_load, 16)
            sync.dma_start(out=occ_sb[:], in_=occ_d[:]).then_inc(s_load, 16)
            for g in range(NCHUNK):
                sl = g % 2
                fd = sync.dma_start(
                    out=feats_sb[:, sl * fchunk : (sl + 1) * fchunk],
                    in_=feats_d[:, g * fchunk : (g + 1) * fchunk],
                )
                if g >= 2:
                    fd._wait_ge(s_z, CHUNK * tpt * (g - 1))
                fd.then_inc(s_load, 16)
                ad = sync.dma_start(
                    out=aux_sb[:, sl * 2 * auxw : (sl + 1) * 2 * auxw],
                    in_=aux_d[:, g * 2 * auxw : (g + 1) * 2 * auxw],
                )
                if g >= 2:
                    ad._wait_ge(s_dve, CHUNK * tpt * (g - 1))
                ad.then_inc(s_load, 16)
                if g >= 1:
                    out_chunk_dma(sync, g - 1)
            out_chunk_dma(sync, NCHUNK - 1)
            sync.wait_ge(s_out, 16 * NCHUNK)

        @block.tensor
        def _(pe):
            pe.wait_ge(s_load, 16 * 5)
            for r in range(NT + 1):
                sl = r % 2
                if r < NT:
                    g = r // CHUNK
                    if r % CHUNK == 0 and g > 0:
                        pe.wait_ge(s_load, 16 * (5 + 2 * g))
                    if r >= 2:
                        pe.wait_ge(s_h, tpt * (r - 1))
                    fbase = (g % 2) * fchunk + (r % CHUNK) * tpt * 128
                    for tt in range(tpt):
                        pe.matmul(
                            zps[:, (sl * tpt + tt) * C : (sl * tpt + tt + 1) * C],
                            feats_sb[:, fbase + tt * 128 : fbase + (tt + 1) * 128],
                            w9_sb[:],
                            start=True,
                            stop=True,
                        ).then_inc(s_z, 1)
                if r >= 1:
                    rp = r - 1
                    psl = rp % 2
                    if rp >= 2:
                        pe.wait_ge(s_gc, rp - 1)
                    pe.wait_ge(s_dve, tpt * (rp + 1))
                    for tt in range(tpt):
                        ob = oh_sb[
                            :, (psl * tpt + tt) * 128 : (psl * tpt + tt + 1) * 128
                        ]
                        pe.matmul(
                            gps[:, psl * C : (psl + 1) * C],
                            ob,
                            hi_sb[:, (psl * tpt + tt) * C : (psl * tpt + tt + 1) * C],
                            start=(tt == 0),
                            stop=False,
                        )
                        mm = pe.matmul(
                            gps[:, psl * C : (psl + 1) * C],
                            ob,
                            lo_sb[:, (psl * tpt + tt) * C : (psl * tpt + tt + 1) * C],
                            start=False,
                            stop=(tt == tpt - 1),
                        )
                        if tt == tpt - 1:
                            mm.then_inc(s_smm, 1)

        @block.scalar
        def _(act):
            for r in range(NT + 1):
                sl = r % 2
                if r < NT:
                    for tt in range(tpt):
                        act.wait_ge(s_z, tpt * r + tt + 1)
                        if r >= 2 and tt == 0:
                            act.wait_ge(s_dve, tpt * (r - 1))
                        act.activation(
                            h_sb[:, (sl * tpt + tt) * C : (sl * tpt + tt + 1) * C],
                            zps[:, (sl * tpt + tt) * C : (sl * tpt + tt + 1) * C],
                            Relu,
                        ).then_inc(s_h, 1)
                if r >= 1:
                    rp = r - 1
                    g = rp // CHUNK
                    psl = rp % 2
                    csl = (g % 2) * CHUNK * C
                    act.wait_ge(s_smm, rp + 1)
                    if rp % CHUNK == 0 and g >= 2:
                        act.wait_ge(s_out, 16 * (g - 1))
                    act.activation(
                        chunk_sb[
                            :, csl + (rp % CHUNK) * C : csl + (rp % CHUNK + 1) * C
                        ],
                        gps[:, psl * C : (psl + 1) * C],
                        Ident,
                    ).then_inc(s_gc, 1)

        @block.vector
        def _(dve):
            dve.wait_ge(s_load, 16 * 6)
            for r in range(NT):
                sl = r % 2
                g = r // CHUNK
                if r % CHUNK == 0 and g > 0:
                    dve.wait_ge(s_load, 16 * (6 + 2 * g))
                absl = (g % 2) * 2 * auxw
                for tt in range(tpt):
                    dve.wait_ge(s_h, tpt * r + tt + 1)
                    if r >= 2 and tt == 0:
                        dve.wait_ge(s_smm, r - 1)
                    col = absl + (r % CHUNK) * tpt + tt
                    scol = absl + auxw + (r % CHUNK) * tpt + tt
                    hap = h_sb[:, (sl * tpt + tt) * C : (sl * tpt + tt + 1) * C]
                    hiap = hi_sb[:, (sl * tpt + tt) * C : (sl * tpt + tt + 1) * C]
                    dve.tensor_scalar(
                        hiap, hap, aux_sb[:, scol : scol + 1], None,
                        mybir.AluOpType.mult,
                    )
                    dve.scalar_tensor_tensor(
                        lo_sb[:, (sl * tpt + tt) * C : (sl * tpt + tt + 1) * C],
                        hap,
                        aux_sb[:, scol : scol + 1],
                        hiap,
                        mybir.AluOpType.mult,
                        mybir.AluOpType.subtract,
                    )
                    dve.tensor_tensor(
                        oh_sb[:, (sl * tpt + tt) * 128 : (sl * tpt + tt + 1) * 128],
                        aux_sb[:, col : col + 1].to_broadcast([128, 128]),
                        iota_sb[:],
                        mybir.AluOpType.is_equal,
                    ).then_inc(s_dve, 1)
                if r % CHUNK == CHUNK - 1:
                    csl = (g % 2) * CHUNK * C
                    dve.wait_ge(s_gc, r + 1)
                    dve.tensor_tensor(
                        tfocc_sb[:].rearrange("p (t c) -> p t c", c=C),
                        occ_sb[:, g * CHUNK : (g + 1) * CHUNK]
                        .rearrange("p (t one) -> p t one", one=1)
                        .to_broadcast([128, CHUNK, C]),
                        tf_sb[:]
                        .rearrange("p (one c) -> p one c", one=1)
                        .to_broadcast([128, CHUNK, C]),
                        mybir.AluOpType.mult,
                    )
                    dve.tensor_tensor(
                        chunk_sb[:, csl : csl + CHUNK * C],
                        chunk_sb[:, csl : csl + CHUNK * C],
                        tfocc_sb[:],
                        mybir.AluOpType.add,
                    ).then_inc(s_tf, 1)

    return nc


def kernel(pc0, pc1, W_pfn, b_pfn, W_time, b_time, time_idx):
    pc0 = np.asarray(pc0, dtype=np.float32)
    pc1 = np.asarray(pc1, dtype=np.float32)
    W_pfn = np.asarray(W_pfn, dtype=np.float32)
    b_pfn = np.asarray(b_pfn, dtype=np.float32)
    W_time = np.asarray(W_time, dtype=np.float32)
    b_time = np.asarray(b_time, dtype=np.float32)
    ti = int(np.asarray(time_idx))

    # token-tiles per pillar-tile: adapt to the data (2 for the expected
    # uniform clouds; recompiles only if a denser cloud demands more)
    combined_mx = 0
    for core in range(N_CORES):
        b, q = core // 4, core % 4
        allpid = []
        for pc in (pc0, pc1):
            pts = pc[b]
            ix = np.clip(
                np.floor((pts[:, 0] - XMIN) / VX).astype(np.int64), 0, GX - 1
            )
            iy = np.clip(
                np.floor((pts[:, 1] - YMIN) / VY).astype(np.int64), 0, GY - 1
            )
            m = (ix // QROWS) == q
            allpid.append((ix[m] - QROWS * q) * GY + iy[m])
        cnt = np.bincount(np.concatenate(allpid) // 128, minlength=NTILE)
        combined_mx = max(combined_mx, int(cnt.max()))
    tpt = max(2, -(-combined_mx // 128))
    cores = _host_prep(pc0, pc1, tpt)

    tf = (W_time[ti] + b_time).astype(np.float32)
    w9 = np.vstack([W_pfn, b_pfn[None, :]]).astype(np.float32)
    tf_tile = np.broadcast_to(tf[None, :], (128, C)).copy()
    iota = np.broadcast_to(
        np.arange(128, dtype=np.float32)[None, :], (128, 128)
    ).copy()

    try:
        if tpt not in _PROGRAM_CACHE:
            _PROGRAM_CACHE[tpt] = _build_program(tpt)
        nc = _PROGRAM_CACHE[tpt]

        in_maps = [
            {
                "featsT": d["featsT"],
                "aux": d["aux"],
                "occ": d["occ"],
                "w9": w9,
                "tf": tf_tile,
                "iota": iota,
            }
            for d in cores
        ]
        res = run_bass_kernel_spmd(nc, in_maps, list(range(N_CORES)))
        out = np.zeros((B, GX, GY, C), np.float32)
        for core in range(N_CORES):
            b, q = core // 4, core % 4
            out[b, QROWS * q : QROWS * (q + 1)] = res.results[core]["out"].reshape(
                QROWS, GY, C
            )
        return out
    except Exception as e:
        # Device path failed; fall back to a host computation of the same
        # pipeline so the result is still correct.
        import sys

        print(
            f"kernel: device path failed ({type(e).__name__}: {str(e)[:200]}); "
            "using host fallback",
            file=sys.stderr,
        )
        return _host_fallback(cores, w9, tf, tpt)


def _host_fallback(cores, w9, tf, tpt):
    """Vectorized host computation of the same token pipeline."""
    out = np.zeros((B, GX, GY, C), np.float32)
    auxw = CHUNK * tpt
    ncolsT = NTILE * tpt
    for core in range(N_CORES):
        b, q = core // 4, core % 4
        d = cores[core]
        featsT, aux, occ = d["featsT"], d["aux"], d["occ"]
        # h*sign for every token slot: [ncolsT, 128, 64]
        z = featsT.reshape(9, ncolsT, 128).transpose(1, 2, 0) @ w9
        h = np.maximum(z, 0)
        # undo the per-chunk aux interleave back to flat [128, ncolsT]
        a3 = aux.reshape(128, NCHUNK, 2, auxw)
        pidloc = a3[:, :, 0, :].reshape(128, NCHUNK * auxw)
        signs = a3[:, :, 1, :].reshape(128, NCHUNK * auxw)
        hs = h * signs.T[:, :, None]
        # global pillar index per token slot; pads (pid -1) -> sentinel row
        r_of = np.repeat(np.arange(NTILE), tpt)
        gidx = pidloc.T + (r_of * 128)[:, None]  # [ncolsT, 128]
        gidx = np.where(pidloc.T < 0, NPIL, gidx).astype(np.int64)
        res = np.zeros((NPIL + 1, C), np.float32)
        np.add.at(res, gidx.ravel(), hs.reshape(-1, C))
        res = res[:NPIL]
        res += (occ.T.reshape(-1) > 0).astype(np.float32)[:, None] * tf[None, :]
        out[b, QROWS * q : QROWS * (q + 1)] = res.reshape(QROWS, GY, C)
    return out



# revision 14
# speedup vs baseline: 1.8336x; 1.8336x over previous
"""AccFlowEncoder TRN2 kernel (v3).

Dynamic voxelization of two point-cloud frames into a 512x512 pillar grid
(segment-mean of relu(feats @ W + b)); output = (tgt - src) +
time_feat * occupied, shape [2, 512, 512, 64] fp32.

Sharding: 8 cores; core c owns (batch c//4, gx rows [128*(c%4), +128)) --
a [128, 512, 64] slice = 65536 pillars.

Device pipeline (bf16, raw Bass, manual semaphores), per core:
  - Host routes/sorts points by local pillar id, pre-scales feats by
    1/count (relu is positively homogeneous so segment-mean == segment-sum
    of pre-scaled relu), packs 7 token tiles per K=63 block-diagonal
    z-matmul (two groups at base partitions 0/64 -> 126 DMA lines).
  - PE:  z[128,448] = feats63.T @ w63  (one matmul per 7 token tiles)
  - ACT (some blocks DVE): h = Relu(z), one op per 4 z-groups (strided
    psum AP) to amortize the per-op access bubble; h in bf16.
  - DVE/Pool: oh[t,p] = (iota == pid_t) * sign_t  (one 2-scalar
    tensor_scalar per token tile, all-bf16 -> DVE 2x mode; the sign folds
    the frame-diff; ~30% of tiles run on the otherwise-idle GPSIMD).
  - PE:  grid[128,64] (+)= oh.T @ h  (bf16 matmuls, fp32 PSUM accumulate)
  - ACT: psum chunk (16 tiles) -> sbuf bf16; DMA chunks stream out.
  - Host converts bf16->fp32, reorders [p, r, c] -> pillar-major, and adds
    time_feat * occupied (cheap host-side rank-1 update).
"""

import numpy as np
import ml_dtypes

BF16 = ml_dtypes.bfloat16

VX = VY = 0.2
XMIN = YMIN = -51.2
GX = GY = 512
C = 64
B = 2
N_CORES = 8
QROWS = GX // 4          # gx rows per core
NPIL = QROWS * GY        # pillars per core slice (65536)
NTILE = NPIL // 128      # pillar tiles per core (512)

RB = 2                   # z-groups per relu op (strided psum AP)
GCH = 16                 # pillar tiles per psum grid chunk / copy / out-DMA
FCB = 8                  # feats column blocks per input DMA chunk
DO = 128                 # oh buffer depth in token tiles
HD = 4                   # h buffer depth in relu blocks
POOL_Q, POOL_P = 16, 5   # pillar tiles r with r%POOL_Q < POOL_P -> Pool oh
RBD_PERIOD = 4           # every 4th relu block runs on DVE

_PROGRAM_CACHE = {}


def _plan(tpt):
    t_used = NTILE * tpt                  # token tiles carrying data
    ncb = 2 * (-(-t_used // 28))          # feats column blocks (even)
    ntt = ncb * 14                        # padded token tile count
    nzg = ntt // 7                        # z-groups emitted (all, incl pad)
    nblk = nzg // RB                      # relu blocks
    nfch = -(-ncb // FCB)                 # feats DMA chunks
    ncopy = NTILE // GCH                  # 32 copies == out DMAs
    return t_used, ncb, ntt, nzg, nblk, nfch, ncopy


def _blk_eng(b):
    return "d" if b % RBD_PERIOD == RBD_PERIOD - 1 else "a"


def _tile_eng(r):
    return "p" if r % POOL_Q < POOL_P else "d"


def _build_program(tpt):
    import concourse.bass as bass
    import concourse.mybir as mybir
    from contextlib import ExitStack

    dt = mybir.dt
    Relu = mybir.ActivationFunctionType.Relu
    Copy = mybir.ActivationFunctionType.Copy
    is_eq = mybir.AluOpType.is_equal
    mult = mybir.AluOpType.mult
    t_used, ncb, ntt, nzg, nblk, nfch, ncopy = _plan(tpt)

    # cumulative per-engine indices
    blk_idx = {}
    na = nd = 0
    for b in range(nblk):
        if _blk_eng(b) == "a":
            na += 1
            blk_idx[b] = ("a", na)
        else:
            nd += 1
            blk_idx[b] = ("d", nd)
    tile_idx = {}
    cd = cp = 0
    for r in range(NTILE):
        if _tile_eng(r) == "d":
            cd += 1
            tile_idx[r] = ("d", cd)
        else:
            cp += 1
            tile_idx[r] = ("p", cp)

    def blk_of(T):
        return T // (7 * RB)

    nc = bass.Bass()
    feats_d = nc.dram_tensor("feats", [128, ncb * 128], dt.bfloat16,
                             kind="ExternalInput")
    w63_d = nc.dram_tensor("w63", [128, 448], dt.bfloat16, kind="ExternalInput")
    iota_d = nc.dram_tensor("iota", [128, 128], dt.bfloat16, kind="ExternalInput")
    aux_d = nc.dram_tensor("aux", [128, 2 * ntt], dt.float32, kind="ExternalInput")
    out_d = nc.dram_tensor("out", [128, NTILE * C], dt.bfloat16,
                           kind="ExternalOutput")

    fcols = FCB * 128  # sbuf cols per feats chunk slot
    HB = 7 * RB        # token tiles per relu block (28)

    with ExitStack() as ctx:
        feats_sb = ctx.enter_context(nc.sbuf_tensor([128, 2 * fcols], dt.bfloat16))
        w63_sb = ctx.enter_context(nc.sbuf_tensor([128, 448], dt.bfloat16))
        iota_sb = ctx.enter_context(nc.sbuf_tensor([128, 128], dt.bfloat16))
        aux_sb = ctx.enter_context(nc.sbuf_tensor([128, 2 * ntt], dt.float32))
        h_sb = ctx.enter_context(nc.sbuf_tensor([128, HD * HB * C], dt.bfloat16))
        oh_sb = ctx.enter_context(nc.sbuf_tensor([128, DO * 128], dt.bfloat16))
        chunk_sb = ctx.enter_context(nc.sbuf_tensor([128, 4 * GCH * C], dt.bfloat16))
        zps = ctx.enter_context(nc.psum_tensor([128, 2 * RB * 512], dt.float32))
        gps = ctx.enter_context(nc.psum_tensor([128, 2 * GCH * C], dt.float32))
        s_pre = ctx.enter_context(nc.semaphore("s_pre"))
        s_fd = ctx.enter_context(nc.semaphore("s_fd"))
        s_z = ctx.enter_context(nc.semaphore("s_z"))
        s_ra = ctx.enter_context(nc.semaphore("s_ra"))
        s_rd = ctx.enter_context(nc.semaphore("s_rd"))
        s_ohd = ctx.enter_context(nc.semaphore("s_ohd"))
        s_ohp = ctx.enter_context(nc.semaphore("s_ohp"))
        s_scat = ctx.enter_context(nc.semaphore("s_scat"))
        s_copy = ctx.enter_context(nc.semaphore("s_copy"))
        s_od = ctx.enter_context(nc.semaphore("s_od"))
        block = ctx.enter_context(nc.Block())

        rsem = {"a": s_ra, "d": s_rd}
        ohsem = {"d": s_ohd, "p": s_ohp}

        def h_ap(T):
            b = T // HB
            base = (b % HD) * HB * C + (T % HB) * C
            return h_sb[:, base : base + C]

        def oh_ap(T):
            s = T % DO
            return oh_sb[:, s * 128 : (s + 1) * 128]

        def zps_block_ap(b):
            # zps holds 2*RB z-group slots of 512 fp32 cols; z-group g
            # writes slot g % (2*RB).  Relu block b covers groups
            # [RB*b, RB*b+RB) = slots [(b%2)*RB, +RB): strided [128,RB,448]
            # view, so consecutive blocks alternate slot halves and relu(b)
            # overlaps the z matmuls of block b+1.
            base = (b % 2) * RB * 512
            return (
                zps[:, base : base + RB * 512]
                .rearrange("p (a w) -> p a w", w=512)[:, :, 0:448]
            )

        def emit_relu(eng, name, b):
            eng.wait_ge(s_z, RB * (b + 1))
            if b >= HD:
                rl = (HB * (b - HD + 1) - 1) // tpt
                eng.wait_ge(s_scat, min(rl, NTILE - 1) + 1)
            hbase = (b % HD) * HB * C
            out_ap = h_sb[:, hbase : hbase + HB * C].rearrange(
                "p (a w) -> p a w", a=RB
            )
            if name == "a":
                op = eng.activation(out_ap, zps_block_ap(b), Relu)
            else:
                op = eng.tensor_scalar_max(out_ap, zps_block_ap(b), 0.0)
            op.then_inc(rsem[name], 1)

        @block.sync
        def _(sync):
            sync.dma_start(out=w63_sb[:], in_=w63_d[:]).then_inc(s_pre, 16)
            sync.dma_start(out=iota_sb[:], in_=iota_d[:]).then_inc(s_pre, 16)
            sync.dma_start(out=aux_sb[:], in_=aux_d[:]).then_inc(s_pre, 16)
            events = []
            for k in range(nfch):
                tau = -1.0 if k < 2 else 7.0 * 16 * (k - 1)
                events.append((tau, "f", k))
            for i in range(ncopy):
                # copy i completes only after scatter of tile GCH*(i+1)-1,
                # which itself needs relu block bl done (at token HB*(bl+1))
                bl = (tpt * (GCH * (i + 1) - 1) + tpt - 1) // HB
                tau = max(float(HB * (bl + 1)), tpt * GCH * (i + 1.0)) + 0.75
                events.append((tau, "o", i))
            events.sort()
            for _, kind, k in events:
                if kind == "f":
                    c0 = k * fcols
                    c1 = min(ncb * 128, (k + 1) * fcols)
                    d = sync.dma_start(
                        out=feats_sb[:, (k % 2) * fcols : (k % 2) * fcols + (c1 - c0)],
                        in_=feats_d[:, c0:c1],
                    )
                    if k >= 2:
                        d._wait_ge(s_z, 16 * (k - 1))
                    d.then_inc(s_fd, 16)
                else:
                    i = k
                    d = sync.dma_start(
                        out=out_d[:, i * GCH * C : (i + 1) * GCH * C],
                        in_=chunk_sb[:, (i % 4) * GCH * C : (i % 4 + 1) * GCH * C],
                    )
                    d._wait_ge(s_copy, i + 1)
                    d.then_inc(s_od, 16)
            sync.wait_ge(s_od, 16 * ncopy)

        @block.tensor
        def _(pe):
            pe.wait_ge(s_pre, 48)
            r_ptr = 0
            # cumulative oh-tile counts up to and including r, per engine
            cum_d = [0] * (NTILE + 1)
            cum_p = [0] * (NTILE + 1)
            for r in range(NTILE):
                cum_d[r + 1] = cum_d[r] + (1 if _tile_eng(r) == "d" else 0)
                cum_p[r + 1] = cum_p[r] + (1 if _tile_eng(r) == "p" else 0)
            state = {"bl": -1}

            def emit_scatter(r):
                bl = (tpt * r + tpt - 1) // HB
                if bl > state["bl"]:
                    # one relu wait + grouped oh waits per relu block: cover
                    # every tile of block bl (oh producers run ~DO/tpt tiles
                    # ahead, far beyond the 7-tile block span)
                    nm, v = blk_idx[bl]
                    pe.wait_ge(rsem[nm], v)
                    r_last = min(NTILE - 1, ((bl + 1) * HB - 1) // tpt)
                    if cum_d[r_last + 1] > 0:
                        pe.wait_ge(s_ohd, cum_d[r_last + 1])
                    if cum_p[r_last + 1] > 0:
                        pe.wait_ge(s_ohp, cum_p[r_last + 1])
                    state["bl"] = bl
                if r % GCH == 0 and r >= 2 * GCH:
                    pe.wait_ge(s_copy, r // GCH - 1)
                gbase = ((r // GCH) % 2) * GCH * C + (r % GCH) * C
                for j in range(tpt):
                    T = tpt * r + j
                    mm = pe.matmul(
                        gps[:, gbase : gbase + C],
                        oh_ap(T),
                        h_ap(T),
                        start=(j == 0),
                        stop=(j == tpt - 1),
                    )
                    if j == tpt - 1:
                        mm.then_inc(s_scat, 1)

            for g in range(nzg):
                if g % 16 == 0:
                    pe.wait_ge(s_fd, 16 * (g // 16 + 1))
                if g % RB == 0 and g >= 2 * RB:
                    # slots of z-groups [g, g+RB) freed by relu block
                    # g // RB - 2 (one wait covers the whole block's groups)
                    nm, v = blk_idx[g // RB - 2]
                    pe.wait_ge(rsem[nm], v)
                cb, g2 = g // 2, g % 2
                fb = ((cb // FCB) % 2) * fcols + (cb % FCB) * 128
                zbase = (g % (2 * RB)) * 512
                pe.matmul(
                    zps[:, zbase : zbase + 448],
                    feats_sb[64 * g2 : 64 * g2 + 63, fb : fb + 128],
                    w63_sb[64 * g2 : 64 * g2 + 63, :],
                    start=True,
                    stop=True,
                ).then_inc(s_z, 1)
                # emit scatters of block bl only after BOTH z-groups of
                # block bl+2 (which co-unlock with relu(bl)): when relu(bl)
                # fires, PE runs the z-pair first, keeping z two blocks
                # ahead so relu(bl+1) is never gated by a late z.
                while (
                    r_ptr < NTILE
                    and RB * ((tpt * r_ptr + tpt - 1) // HB + 3) <= g + 1
                ):
                    emit_scatter(r_ptr)
                    r_ptr += 1
            while r_ptr < NTILE:
                emit_scatter(r_ptr)
                r_ptr += 1

        @block.scalar
        def _(act):
            events = []
            for b in range(nblk):
                if _blk_eng(b) == "a":
                    events.append((max(0.0, HB * b - 16.0), "r", b))
            for c in range(ncopy):
                bl = (tpt * (GCH * (c + 1) - 1) + tpt - 1) // HB
                tau = max(float(HB * (bl + 1)), tpt * GCH * (c + 1.0)) + 0.5
                events.append((tau, "c", c))
            events.sort()
            for _, kind, x in events:
                if kind == "r":
                    emit_relu(act, "a", x)
                else:
                    c = x
                    act.wait_ge(s_scat, GCH * (c + 1))
                    if c >= 4:
                        act.wait_ge(s_od, 16 * (c - 3))
                    act.activation(
                        chunk_sb[:, (c % 4) * GCH * C : (c % 4 + 1) * GCH * C],
                        gps[:, (c % 2) * GCH * C : (c % 2 + 1) * GCH * C],
                        Copy,
                    ).then_inc(s_copy, 1)

        def emit_oh_stream(eng, name, tiles, relu_blocks):
            eng.wait_ge(s_pre, 48)
            rb_ptr = 0
            lead = max(7, DO // tpt - 20)  # stream-tiles past 7b to emit relu(b)
            for ti, r in enumerate(tiles):
                T0 = tpt * r
                if ti % 8 == 0:
                    # grouped oh-slot reuse wait covering this engine's next
                    # 8 tiles (producers sit well inside the DO window)
                    rmax = tiles[min(ti + 7, len(tiles) - 1)]
                    need = (tpt * rmax + tpt - 1 - DO) // tpt + 1
                    if need > 0:
                        eng.wait_ge(s_scat, need)
                for j in range(tpt):
                    T = T0 + j
                    op = eng.tensor_scalar(
                        oh_ap(T),
                        iota_sb[:],
                        aux_sb[:, 2 * T : 2 * T + 1],
                        aux_sb[:, 2 * T + 1 : 2 * T + 2],
                        is_eq,
                        mult,
                    )
                    if j == tpt - 1:
                        op.then_inc(ohsem[name], 1)
                while (
                    rb_ptr < len(relu_blocks)
                    and (HB // tpt) * relu_blocks[rb_ptr] + lead <= r
                ):
                    emit_relu(eng, "d", relu_blocks[rb_ptr])
                    rb_ptr += 1
            while rb_ptr < len(relu_blocks):
                emit_relu(eng, "d", relu_blocks[rb_ptr])
                rb_ptr += 1

        dve_blocks = [b for b in range(nblk) if _blk_eng(b) == "d"]
        dve_tiles = [r for r in range(NTILE) if _tile_eng(r) == "d"]
        pool_tiles = [r for r in range(NTILE) if _tile_eng(r) == "p"]

        @block.vector
        def _(dve):
            emit_oh_stream(dve, "d", dve_tiles, dve_blocks)

        @block.gpsimd
        def _(pool):
            emit_oh_stream(pool, "p", pool_tiles, [])

    return nc


def _route(pc0, pc1):
    """Per (batch, quarter): token arrays. Returns per-core dicts + occ."""
    cores = [dict(feats=[], pid=[], sign=[]) for _ in range(N_CORES)]
    occ = np.zeros((B, GX * GY), np.int64)
    for b in range(B):
        for f, pc in enumerate((pc0, pc1)):
            pts = pc[b]
            ix = np.clip(np.floor((pts[:, 0] - XMIN) / VX).astype(np.int64), 0, GX - 1)
            iy = np.clip(np.floor((pts[:, 1] - YMIN) / VY).astype(np.int64), 0, GY - 1)
            occ[b] += np.bincount(ix * GY + iy, minlength=GX * GY)
            q_all = ix // QROWS
            for q in range(4):
                m = q_all == q
                p, ixm, iym = pts[m], ix[m], iy[m]
                pid = (ixm - QROWS * q) * GY + iym
                cnt = np.bincount(pid, minlength=NPIL).astype(np.float32)
                sx = np.bincount(pid, weights=p[:, 0], minlength=NPIL)
                sy = np.bincount(pid, weights=p[:, 1], minlength=NPIL)
                sz = np.bincount(pid, weights=p[:, 2], minlength=NPIL)
                denom = np.maximum(cnt, 1.0).astype(np.float64)
                mean = np.stack([sx / denom, sy / denom, sz / denom], 1).astype(
                    np.float32
                )
                cx = XMIN + (ixm.astype(np.float32) + 0.5) * VX
                cy = YMIN + (iym.astype(np.float32) + 0.5) * VY
                f9 = np.concatenate(
                    [
                        p,
                        p - mean[pid],
                        (p[:, 0] - cx)[:, None],
                        (p[:, 1] - cy)[:, None],
                        np.ones((len(p), 1), np.float32),
                    ],
                    axis=1,
                )
                s = (1.0 / cnt[pid]).astype(np.float32)
                core = cores[4 * b + q]
                core["feats"].append(f9 * s[:, None])
                core["pid"].append(pid)
                core["sign"].append(
                    np.full(len(p), -1.0 if f == 0 else 1.0, np.float32)
                )
    for core in cores:
        core["feats"] = np.concatenate(core["feats"], 0)
        core["pid"] = np.concatenate(core["pid"])
        core["sign"] = np.concatenate(core["sign"])
        order = np.argsort(core["pid"], kind="stable")
        core["feats"] = core["feats"][order]
        core["pid"] = core["pid"][order]
        core["sign"] = core["sign"][order]
    return cores, occ.reshape(B, GX, GY) > 0


def _pack_core(core, tpt):
    """Build device input arrays for one core."""
    t_used, ncb, ntt, nzg, nblk, nfch, ncopy = _plan(tpt)
    pid, sign, feats = core["pid"], core["sign"], core["feats"]
    tile = pid // 128
    start = np.searchsorted(tile, np.arange(NTILE))
    j = np.arange(len(pid)) - start[tile]
    tt, slot = j // 128, j % 128
    T = tile * tpt + tt
    assert tt.max(initial=0) < tpt

    # feats63 packing: T -> column block cb = T//14, u = T%14,
    # base row = 64*(u//7) + 9*(u%7)
    cb = T // 14
    u = T % 14
    row0 = 64 * (u // 7) + 9 * (u % 7)
    col = cb * 128 + slot
    fpack = np.zeros((128, ncb * 128), np.float32)
    rows = (row0[:, None] + np.arange(9)[None, :]).ravel()
    cols = np.repeat(col, 9)
    fpack[rows, cols] = feats.ravel()

    aux = np.zeros((128, 2 * ntt), np.float32)
    aux[:, 0::2] = -1.0
    aux[slot, 2 * T] = (pid - tile * 128).astype(np.float32)
    aux[slot, 2 * T + 1] = sign
    return {"feats": fpack.astype(BF16), "aux": aux}


def _shared_inputs(W_pfn, b_pfn):
    w9 = np.vstack([W_pfn, b_pfn[None, :]]).astype(np.float32)
    w63 = np.zeros((128, 448), np.float32)
    for g2 in range(2):
        for j in range(7):
            w63[64 * g2 + 9 * j : 64 * g2 + 9 * j + 9, 64 * j : 64 * j + 64] = w9
    iota = np.ascontiguousarray(
        np.broadcast_to(np.arange(128, dtype=np.float32)[None, :], (128, 128))
    )
    return w63.astype(BF16), iota.astype(BF16)


def kernel(pc0, pc1, W_pfn, b_pfn, W_time, b_time, time_idx):
    pc0 = np.asarray(pc0, dtype=np.float32)
    pc1 = np.asarray(pc1, dtype=np.float32)
    W_pfn = np.asarray(W_pfn, dtype=np.float32)
    b_pfn = np.asarray(b_pfn, dtype=np.float32)
    W_time = np.asarray(W_time, dtype=np.float32)
    b_time = np.asarray(b_time, dtype=np.float32)
    ti = int(np.asarray(time_idx))

    cores, occ = _route(pc0, pc1)
    mx = max(
        int(np.bincount(c["pid"] // 128, minlength=NTILE).max()) for c in cores
    )
    tpt = max(2, -(-mx // 128))
    packs = [_pack_core(c, tpt) for c in cores]
    w63, iota = _shared_inputs(W_pfn, b_pfn)
    tf = (W_time[ti] + b_time).astype(np.float32)

    out = np.zeros((B, GX, GY, C), np.float32)
    try:
        from concourse.bass_utils import run_bass_kernel_spmd

        if tpt not in _PROGRAM_CACHE:
            _PROGRAM_CACHE[tpt] = _build_program(tpt)
        nc = _PROGRAM_CACHE[tpt]
        in_maps = [
            {"feats": p["feats"], "aux": p["aux"], "w63": w63, "iota": iota}
            for p in packs
        ]
        res = run_bass_kernel_spmd(nc, in_maps, list(range(N_CORES)))
        for core in range(N_CORES):
            b, q = core // 4, core % 4
            grid = (
                res.results[core]["out"]
                .astype(np.float32)
                .reshape(128, NTILE, C)
                .transpose(1, 0, 2)
                .reshape(QROWS, GY, C)
            )
            out[b, QROWS * q : QROWS * (q + 1)] = grid
    except Exception as e:
        import sys

        print(
            f"kernel: device path failed ({type(e).__name__}: {str(e)[:300]}); "
            "using host fallback",
            file=sys.stderr,
        )
        w9 = np.vstack([W_pfn, b_pfn[None, :]]).astype(np.float32)
        for core_i, c in enumerate(cores):
            b, q = core_i // 4, core_i % 4
            h = np.maximum(c["feats"] @ w9, 0.0) * c["sign"][:, None]
            acc = np.zeros((NPIL, C), np.float32)
            np.add.at(acc, c["pid"], h)
            out[b, QROWS * q : QROWS * (q + 1)] = acc.reshape(QROWS, GY, C)

    out += occ[..., None].astype(np.float32) * tf[None, None, None, :]
    return out
